# revision 8
# baseline (speedup 1.0000x reference)
"""Trainium2 Bass kernel for nn_DissipativeSimplestRINN.

Reference computation (per time step t, batch B):
    v_const = x @ Cv_T + y @ Dvy_T
    w = fixed_point(w -> tanh(v_const + w @ Dvw_T), 30 iters, w0 = 0)
    u = x @ Cu_T + w @ Duw_T + y @ Duy_T
    xdot = x @ A_T + w @ Bw_T + y @ By_T
    x <- x + DT * xdot
Outputs: concat([u, log_stds], -1) of shape [B, T, 16], and x_final [B, STATE].

The fixed-point map is a strong contraction (||Dvw||_2 ~ 0.28), so it
converges to the fp32 fixed point in ~9 iterations; iterations beyond that
are identity at fp32 precision.  We run 1 free iteration (tanh(v_const)),
NB bf16 matmul iterations, then NF fp32 matmul iterations; host-side numpy
emulation of this scheme matches the full 30-iteration reference to ~1.6e-7
relative error (and the hardware run matches the reference to ~2.5e-7).

Sharding: data-parallel over batch across 8 cores (256 rows/core), with all
state kept transposed on-chip ([feature, batch] layout) so every matmul has
the small controller matrix as the stationary operand and batch as the
moving free dim.  Each core's 256-row batch is processed as 2 interleaved
chunks of 128 so tensor/vector/scalar engines pipeline across chunks.
"""

import contextlib

import numpy as np
import ml_dtypes

from concourse import bacc, bass, tile, mybir
from concourse.bass_utils import run_bass_kernel_spmd

N_CORES = 8
B = 2048
T_STEPS = 64
INP, STATE, NONLIN, OUT = 32, 64, 128, 8
BC = B // N_CORES          # batch per core (256)
NCH = 2                    # chunks per core
CW = BC // NCH             # chunk width (128)
Z = STATE + INP            # packed [x; y] feature dim (96)
OD = STATE + OUT           # packed [xdot | u] output dim (72)
NB = 4                     # bf16 fixed-point matmul iterations
NF = 4                     # fp32 fixed-point matmul iterations
U_FLUSH = 8                # steps of u staged in SBUF per DMA flush
DT = 0.01
LOG_STD = -1.6094379124341003

_NC_CACHE = {}


def _build_nc(t_steps=T_STEPS, reps=1):
    f32 = mybir.dt.float32
    bf16 = mybir.dt.bfloat16
    nc = bacc.Bacc("TRN2", target_bir_lowering=False)

    obs_t = nc.dram_tensor("obs_t", [t_steps, NCH, INP, CW], f32, kind="ExternalInput")
    z_init = nc.dram_tensor("z_init", [NCH, Z, CW], f32, kind="ExternalInput")
    wv = nc.dram_tensor("wv", [Z, NONLIN], f32, kind="ExternalInput")
    woz = nc.dram_tensor("woz", [Z, OD], f32, kind="ExternalInput")
    wow = nc.dram_tensor("wow", [NONLIN, OD], f32, kind="ExternalInput")
    dvw = nc.dram_tensor("dvw", [NONLIN, NONLIN], f32, kind="ExternalInput")
    dvwb = nc.dram_tensor("dvwb", [NONLIN, NONLIN], bf16, kind="ExternalInput")
    out_u = nc.dram_tensor("out_u", [OUT, t_steps, BC], f32, kind="ExternalOutput")
    out_x = nc.dram_tensor("out_x", [NCH, STATE, CW], f32, kind="ExternalOutput")

    tanh = mybir.ActivationFunctionType.Tanh

    with tile.TileContext(nc) as tc:
        with (
            tc.tile_pool(name="const", bufs=1) as cpool,
            tc.tile_pool(name="state", bufs=1) as spool,
            tc.tile_pool(name="stmp", bufs=4) as tpool,
            tc.tile_pool(name="ppv", bufs=2, space="PSUM") as ppv,
            tc.tile_pool(name="ppw", bufs=4, space="PSUM") as ppw,
            tc.tile_pool(name="ppo", bufs=2, space="PSUM") as ppo,
        ):
            # --- weights into SBUF (resident) ---
            wv_sb = cpool.tile([Z, NONLIN], f32, tag="wv", name="wv_sb")
            woz_sb = cpool.tile([Z, OD], f32, tag="woz", name="woz_sb")
            wow_sb = cpool.tile([NONLIN, OD], f32, tag="wow", name="wow_sb")
            dvw_sb = cpool.tile([NONLIN, NONLIN], f32, tag="dvw", name="dvw_sb")
            dvwb_sb = cpool.tile([NONLIN, NONLIN], bf16, tag="dvwb", name="dvwb_sb")
            nc.sync.dma_start(out=wv_sb[:], in_=wv[:])
            nc.sync.dma_start(out=woz_sb[:], in_=woz[:])
            nc.sync.dma_start(out=wow_sb[:], in_=wow[:])
            nc.sync.dma_start(out=dvw_sb[:], in_=dvw[:])
            nc.sync.dma_start(out=dvwb_sb[:], in_=dvwb[:])

            # --- persistent state tiles (per chunk) ---
            # z = [xT; yT]: [96, CW]; double-buffered across steps.
            z = [
                [spool.tile([Z, CW], f32, tag=f"z{bi}{c}", name=f"z{bi}{c}")
                 for c in range(NCH)]
                for bi in range(2)
            ]
            w = [spool.tile([NONLIN, CW], f32, tag=f"w{c}", name=f"w{c}")
                 for c in range(NCH)]
            wb = [spool.tile([NONLIN, CW], bf16, tag=f"wb{c}", name=f"wb{c}")
                  for c in range(NCH)]
            vsb = [spool.tile([NONLIN, CW], f32, tag=f"vsb{c}", name=f"vsb{c}")
                   for c in range(NCH)]
            us = [spool.tile([OUT, U_FLUSH * BC], f32, tag=f"us{bi}", name=f"us{bi}")
                  for bi in range(2)]

            loop_cm = tc.For_i(0, reps) if reps > 1 else contextlib.nullcontext()
            with loop_cm:
                for c in range(NCH):
                    nc.sync.dma_start(out=z[0][c][:], in_=z_init[c])

                for t in range(t_steps):
                    cur, nxt = t % 2, (t + 1) % 2
                    ustage = us[(t // U_FLUSH) % 2]
                    for c in range(NCH):
                        # v_const^T = wv^T @ z  (fp32)
                        pv = ppv.tile([NONLIN, CW], f32, tag="pv", name="pv")
                        nc.tensor.matmul(pv[:], wv_sb[:], z[cur][c][:],
                                         start=True, stop=True)
                        nc.vector.tensor_copy(vsb[c][:], pv[:])
                        first_out = wb[c] if NB > 0 else w[c]
                        nc.scalar.activation(first_out[:], pv[:], tanh)

                        # bf16 fixed-point iterations
                        for i in range(NB):
                            pw = ppw.tile([NONLIN, CW], f32, tag="pw", name="pw")
                            nc.tensor.matmul(pw[:], dvwb_sb[:], wb[c][:],
                                             start=True, stop=True)
                            s = tpool.tile([NONLIN, CW], f32, tag="s", name="s")
                            nc.vector.tensor_add(s[:], pw[:], vsb[c][:])
                            ot = w[c] if i == NB - 1 else wb[c]
                            nc.scalar.activation(ot[:], s[:], tanh)

                        # fp32 fixed-point iterations
                        for i in range(NF):
                            pw = ppw.tile([NONLIN, CW], f32, tag="pw", name="pw")
                            nc.tensor.matmul(pw[:], dvw_sb[:], w[c][:],
                                             start=True, stop=True)
                            s = tpool.tile([NONLIN, CW], f32, tag="s", name="s")
                            nc.vector.tensor_add(s[:], pw[:], vsb[c][:])
                            nc.scalar.activation(w[c][:], s[:], tanh)

                        # [xdot*DT | u]^T = woz^T @ z + wow^T @ w  (fp32)
                        po = ppo.tile([OD, CW], f32, tag="po", name="po")
                        nc.tensor.matmul(po[:], woz_sb[:], z[cur][c][:],
                                         start=True, stop=False)
                        nc.tensor.matmul(po[:], wow_sb[:], w[c][:],
                                         start=False, stop=True)

                        # x <- x + DT*xdot   (DT pre-folded into woz/wow)
                        nc.vector.tensor_add(
                            z[nxt][c][0:STATE, :], z[cur][c][0:STATE, :],
                            po[0:STATE, :]
                        )
                        # stage u
                        col0 = (t % U_FLUSH) * BC + c * CW
                        nc.scalar.copy(ustage[:, col0 : col0 + CW],
                                       po[STATE:OD, :])

                    # prefetch next y into z[nxt]
                    if t + 1 < t_steps:
                        for c in range(NCH):
                            nc.sync.dma_start(out=z[nxt][c][STATE:Z, :],
                                              in_=obs_t[t + 1, c])
                    # flush staged u
                    if t % U_FLUSH == U_FLUSH - 1:
                        nc.sync.dma_start(
                            out=out_u[:, t - (U_FLUSH - 1) : t + 1, :],
                            in_=ustage[:]
                        )

                for c in range(NCH):
                    nc.sync.dma_start(out=out_x[c],
                                      in_=z[t_steps % 2][c][0:STATE, :])

    nc.finalize()
    return nc


def _get_nc(t_steps=T_STEPS, reps=1):
    key = (t_steps, reps)
    if key not in _NC_CACHE:
        _NC_CACHE[key] = _build_nc(t_steps, reps)
    return _NC_CACHE[key]


def _prep_in_maps(inputs, t_steps=T_STEPS):
    obs = np.ascontiguousarray(np.asarray(inputs["obs"], dtype=np.float32))
    x0 = np.ascontiguousarray(np.asarray(inputs["x0"], dtype=np.float32))
    g = {k: np.asarray(inputs[k], dtype=np.float32) for k in
         ("A_T", "Bw_T", "By_T", "Cv_T", "Dvw_T", "Dvy_T", "Cu_T", "Duw_T", "Duy_T")}
    dt = np.float32(DT)

    wv = np.concatenate([g["Cv_T"], g["Dvy_T"]], axis=0)                     # [96,128]
    woz = np.concatenate(
        [
            np.concatenate([dt * g["A_T"], g["Cu_T"]], axis=1),              # [64,72]
            np.concatenate([dt * g["By_T"], g["Duy_T"]], axis=1),            # [32,72]
        ],
        axis=0,
    )                                                                        # [96,72]
    wow = np.concatenate([dt * g["Bw_T"], g["Duw_T"]], axis=1)               # [128,72]
    dvw = np.ascontiguousarray(g["Dvw_T"])
    dvwb = dvw.astype(ml_dtypes.bfloat16)
    wv = np.ascontiguousarray(wv)
    woz = np.ascontiguousarray(woz)
    wow = np.ascontiguousarray(wow)

    in_maps = []
    for i in range(N_CORES):
        base = i * BC
        o = obs[base : base + BC, :t_steps]                 # [BC, t, INP]
        o = o.reshape(NCH, CW, t_steps, INP).transpose(2, 0, 3, 1)  # [t,NCH,INP,CW]
        o = np.ascontiguousarray(o)
        xx = x0[base : base + BC].reshape(NCH, CW, STATE).transpose(0, 2, 1)
        zi = np.concatenate([xx, o[0]], axis=1)             # [NCH, Z, CW]
        in_maps.append(
            {
                "obs_t": o,
                "z_init": np.ascontiguousarray(zi),
                "wv": wv,
                "woz": woz,
                "wow": wow,
                "dvw": dvw,
                "dvwb": dvwb,
            }
        )
    return in_maps


def _run(inputs, t_steps=T_STEPS, trace=False, reps=1, **kwargs):
    nc = _get_nc(t_steps, reps)
    in_maps = _prep_in_maps(inputs, t_steps)
    res = run_bass_kernel_spmd(nc, in_maps, list(range(N_CORES)), trace=trace, **kwargs)
    u_cores, x_cores = [], []
    for i in range(N_CORES):
        ou = np.asarray(res.results[i]["out_u"])            # [OUT, t, BC]
        u_cores.append(ou.transpose(2, 1, 0))               # [BC, t, OUT]
        ox = np.asarray(res.results[i]["out_x"])            # [NCH, STATE, CW]
        x_cores.append(ox.transpose(0, 2, 1).reshape(BC, STATE))
    u = np.concatenate(u_cores, axis=0)
    x = np.concatenate(x_cores, axis=0)
    out = np.empty((B, t_steps, 2 * OUT), np.float32)
    out[..., :OUT] = u
    out[..., OUT:] = np.float32(LOG_STD)
    return (out, x), res


def kernel(**inputs):
    (out, x), _ = _run(inputs)
    return out, x


# revision 11
# speedup vs baseline: 2.8165x; 2.8165x over previous
"""Trainium2 Bass kernel for nn_DissipativeSimplestRINN.

Reference computation (per time step t, batch B):
    v_const = x @ Cv_T + y @ Dvy_T
    w = fixed_point(w -> tanh(v_const + w @ Dvw_T), 30 iters, w0 = 0)
    u = x @ Cu_T + w @ Duw_T + y @ Duy_T
    xdot = x @ A_T + w @ Bw_T + y @ By_T
    x <- x + DT * xdot
Outputs: concat([u, log_stds], -1) of shape [B, T, 16], and x_final [B, STATE].

The fixed-point map is a strong contraction (||Dvw||_2 ~ 0.28), so it
converges to the fp32 fixed point in ~9 iterations; iterations beyond that
are identity at fp32 precision.  We run 1 free iteration (tanh(v_const)),
NB bf16 matmul iterations, then NF fp32 matmul iterations; host-side numpy
emulation of this scheme matches the full 30-iteration reference to ~1.6e-7
relative error (and the hardware run matches the reference to ~2.5e-7).

Sharding: data-parallel over batch across 8 cores (256 rows/core), with all
state kept transposed on-chip ([feature, batch] layout) so every matmul has
the small controller matrix as the stationary operand and batch as the
moving free dim.  Each core's 256-row batch is processed as 2 interleaved
chunks of 128 so tensor/vector/scalar engines pipeline across chunks.
"""

import contextlib

import numpy as np
import ml_dtypes

from concourse import bacc, bass, tile, mybir
from concourse.bass_utils import run_bass_kernel_spmd

N_CORES = 8
B = 2048
T_STEPS = 64
INP, STATE, NONLIN, OUT = 32, 64, 128, 8
BC = B // N_CORES          # batch per core (256)
NCH = 2                    # chunks per core
CW = BC // NCH             # chunk width (128)
Z = STATE + INP            # packed [x; y] feature dim (96)
OD = STATE + OUT           # packed [xdot | u] output dim (72)
NB = 4                     # bf16 fixed-point matmul iterations
NF = 3                     # fp32 fixed-point matmul iterations
U_FLUSH = 8                # steps of u staged in SBUF per DMA flush
DT = 0.01
LOG_STD = -1.6094379124341003

_NC_CACHE = {}
_IDF = np.eye(NONLIN, dtype=np.float32)
_IDB = np.eye(NONLIN).astype(ml_dtypes.bfloat16)


def _build_nc(t_steps=T_STEPS, reps=1):
    f32 = mybir.dt.float32
    bf16 = mybir.dt.bfloat16
    nc = bacc.Bacc("TRN2", target_bir_lowering=False)

    obs_t = nc.dram_tensor("obs_t", [t_steps, NCH, INP, CW], f32, kind="ExternalInput")
    z_init = nc.dram_tensor("z_init", [NCH, Z, CW], f32, kind="ExternalInput")
    wv = nc.dram_tensor("wv", [Z, NONLIN], f32, kind="ExternalInput")
    woz = nc.dram_tensor("woz", [Z, OD], f32, kind="ExternalInput")
    wow = nc.dram_tensor("wow", [NONLIN, OD], f32, kind="ExternalInput")
    dvw = nc.dram_tensor("dvw", [NONLIN, NONLIN], f32, kind="ExternalInput")
    dvwb = nc.dram_tensor("dvwb", [NONLIN, NONLIN], bf16, kind="ExternalInput")
    idf = nc.dram_tensor("idf", [NONLIN, NONLIN], f32, kind="ExternalInput")
    idb = nc.dram_tensor("idb", [NONLIN, NONLIN], bf16, kind="ExternalInput")
    out_u = nc.dram_tensor("out_u", [OUT, t_steps, BC], f32, kind="ExternalOutput")
    out_x = nc.dram_tensor("out_x", [NCH, STATE, CW], f32, kind="ExternalOutput")

    tanh = mybir.ActivationFunctionType.Tanh

    with tile.TileContext(nc) as tc:
        with (
            tc.tile_pool(name="const", bufs=1) as cpool,
            tc.tile_pool(name="state", bufs=1) as spool,
            tc.tile_pool(name="ppw", bufs=6, space="PSUM") as ppw,
            tc.tile_pool(name="ppo", bufs=2, space="PSUM") as ppo,
        ):
            # --- weights into SBUF (resident) ---
            wv_sb = cpool.tile([Z, NONLIN], f32, tag="wv", name="wv_sb")
            woz_sb = cpool.tile([Z, OD], f32, tag="woz", name="woz_sb")
            wow_sb = cpool.tile([NONLIN, OD], f32, tag="wow", name="wow_sb")
            dvw_sb = cpool.tile([NONLIN, NONLIN], f32, tag="dvw", name="dvw_sb")
            dvwb_sb = cpool.tile([NONLIN, NONLIN], bf16, tag="dvwb", name="dvwb_sb")
            nc.sync.dma_start(out=wv_sb[:], in_=wv[:])
            nc.sync.dma_start(out=woz_sb[:], in_=woz[:])
            nc.sync.dma_start(out=wow_sb[:], in_=wow[:])
            nc.sync.dma_start(out=dvw_sb[:], in_=dvw[:])
            nc.sync.dma_start(out=dvwb_sb[:], in_=dvwb[:])
            idf_sb = cpool.tile([NONLIN, NONLIN], f32, tag="idf", name="idf_sb")
            idb_sb = cpool.tile([NONLIN, NONLIN], bf16, tag="idb", name="idb_sb")
            nc.sync.dma_start(out=idf_sb[:], in_=idf[:])
            nc.sync.dma_start(out=idb_sb[:], in_=idb[:])

            # --- persistent state tiles (per chunk) ---
            # z = [xT; yT]: [96, CW]; double-buffered across steps.
            z = [
                [spool.tile([Z, CW], f32, tag=f"z{bi}{c}", name=f"z{bi}{c}")
                 for c in range(NCH)]
                for bi in range(2)
            ]
            w = [spool.tile([NONLIN, CW], f32, tag=f"w{c}", name=f"w{c}")
                 for c in range(NCH)]
            wb = [spool.tile([NONLIN, CW], bf16, tag=f"wb{c}", name=f"wb{c}")
                  for c in range(NCH)]
            vsb = [spool.tile([NONLIN, CW], f32, tag=f"vsb{c}", name=f"vsb{c}")
                   for c in range(NCH)]
            vsbb = [spool.tile([NONLIN, CW], bf16, tag=f"vsbb{c}", name=f"vsbb{c}")
                    for c in range(NCH)]
            us = [spool.tile([OUT, U_FLUSH * BC], f32, tag=f"us{bi}", name=f"us{bi}")
                  for bi in range(2)]

            loop_cm = tc.For_i(0, reps) if reps > 1 else contextlib.nullcontext()
            with loop_cm:
                for c in range(NCH):
                    nc.sync.dma_start(out=z[0][c][:], in_=z_init[c])

                add_op = mybir.AluOpType.add
                NIT = NB + NF          # matmul iterations (iter 0 fused into pv)
                for t in range(t_steps):
                    cur, nxt = t % 2, (t + 1) % 2
                    ustage = us[(t // U_FLUSH) % 2]
                    # --- v_const phase ---
                    pv = [None] * NCH
                    for c in range(NCH):
                        pv[c] = ppw.tile([NONLIN, CW], f32, tag="pw", name="pv")
                        nc.tensor.matmul(pv[c][:], wv_sb[:], z[cur][c][:],
                                         start=True, stop=False,
                                         skip_group_check=True)
                    for c in range(NCH):
                        nc.vector.tensor_copy(vsb[c][:], pv[c][:])
                    for c in range(NCH):
                        # bf16 copy of v_const for the bf16 identity-mm
                        nc.vector.tensor_copy(vsbb[c][:], vsb[c][:])
                    for c in range(NCH):
                        first_out = wb[c] if NB > 0 else w[c]
                        nc.scalar.activation(first_out[:], pv[c][:], tanh)

                    # --- fixed-point iterations ---
                    # iteration i: psum := v_const (identity mm, prefetched
                    # during iteration i-1's tanh) += D @ w_i; tanh -> w_{i+1}.
                    # Iteration 0 accumulates directly onto pv (v_const psum).
                    pw = pv
                    pwn = [None] * NCH
                    for i in range(NIT):
                        lhs = dvwb_sb if i < NB else dvw_sb
                        rin = wb if i < NB else w
                        out_t = wb if i < NB - 1 else w
                        for c in range(NCH):
                            nc.tensor.matmul(pw[c][:], lhs[:], rin[c][:],
                                             start=False, stop=True,
                                             skip_group_check=True)
                        if i + 1 < NIT:
                            # prefetch next iteration's v_const into fresh psum
                            nxt_bf = (i + 1) < NB
                            idm = idb_sb if nxt_bf else idf_sb
                            vsrc = vsbb if nxt_bf else vsb
                            for c in range(NCH):
                                pwn[c] = ppw.tile([NONLIN, CW], f32,
                                                  tag="pw", name="pw")
                                nc.tensor.matmul(pwn[c][:], idm[:], vsrc[c][:],
                                                 start=True, stop=False,
                                                 skip_group_check=True)
                        for c in range(NCH):
                            nc.scalar.activation(out_t[c][:], pw[c][:], tanh)
                        pw, pwn = pwn, [None] * NCH

                    # --- output phase [xdot*DT | u] ---
                    po = [None] * NCH
                    for c in range(NCH):
                        po[c] = ppo.tile([OD, CW], f32, tag="po", name="po")
                        nc.tensor.matmul(po[c][:], woz_sb[:], z[cur][c][:],
                                         start=True, stop=False,
                                         skip_group_check=True)
                    for c in range(NCH):
                        nc.tensor.matmul(po[c][:], wow_sb[:], w[c][:],
                                         start=False, stop=True,
                                         skip_group_check=True)
                    for c in range(NCH):
                        # x <- x + DT*xdot   (DT pre-folded into woz/wow)
                        nc.vector.tensor_add(
                            z[nxt][c][0:STATE, :], z[cur][c][0:STATE, :],
                            po[c][0:STATE, :]
                        )
                    for c in range(NCH):
                        col0 = (t % U_FLUSH) * BC + c * CW
                        nc.scalar.copy(ustage[:, col0 : col0 + CW],
                                       po[c][STATE:OD, :])

                    # prefetch next y into z[nxt]
                    if t + 1 < t_steps:
                        for c in range(NCH):
                            nc.sync.dma_start(out=z[nxt][c][STATE:Z, :],
                                              in_=obs_t[t + 1, c])
                    # flush staged u
                    if t % U_FLUSH == U_FLUSH - 1:
                        nc.sync.dma_start(
                            out=out_u[:, t - (U_FLUSH - 1) : t + 1, :],
                            in_=ustage[:]
                        )

                for c in range(NCH):
                    nc.sync.dma_start(out=out_x[c],
                                      in_=z[t_steps % 2][c][0:STATE, :])

    nc.finalize()
    return nc


def _get_nc(t_steps=T_STEPS, reps=1):
    key = (t_steps, reps)
    if key not in _NC_CACHE:
        _NC_CACHE[key] = _build_nc(t_steps, reps)
    return _NC_CACHE[key]


def _prep_in_maps(inputs, t_steps=T_STEPS):
    obs = np.ascontiguousarray(np.asarray(inputs["obs"], dtype=np.float32))
    x0 = np.ascontiguousarray(np.asarray(inputs["x0"], dtype=np.float32))
    g = {k: np.asarray(inputs[k], dtype=np.float32) for k in
         ("A_T", "Bw_T", "By_T", "Cv_T", "Dvw_T", "Dvy_T", "Cu_T", "Duw_T", "Duy_T")}
    dt = np.float32(DT)

    wv = np.concatenate([g["Cv_T"], g["Dvy_T"]], axis=0)                     # [96,128]
    woz = np.concatenate(
        [
            np.concatenate([dt * g["A_T"], g["Cu_T"]], axis=1),              # [64,72]
            np.concatenate([dt * g["By_T"], g["Duy_T"]], axis=1),            # [32,72]
        ],
        axis=0,
    )                                                                        # [96,72]
    wow = np.concatenate([dt * g["Bw_T"], g["Duw_T"]], axis=1)               # [128,72]
    dvw = np.ascontiguousarray(g["Dvw_T"])
    dvwb = dvw.astype(ml_dtypes.bfloat16)
    wv = np.ascontiguousarray(wv)
    woz = np.ascontiguousarray(woz)
    wow = np.ascontiguousarray(wow)

    in_maps = []
    for i in range(N_CORES):
        base = i * BC
        o = obs[base : base + BC, :t_steps]                 # [BC, t, INP]
        o = o.reshape(NCH, CW, t_steps, INP).transpose(2, 0, 3, 1)  # [t,NCH,INP,CW]
        o = np.ascontiguousarray(o)
        xx = x0[base : base + BC].reshape(NCH, CW, STATE).transpose(0, 2, 1)
        zi = np.concatenate([xx, o[0]], axis=1)             # [NCH, Z, CW]
        in_maps.append(
            {
                "obs_t": o,
                "z_init": np.ascontiguousarray(zi),
                "wv": wv,
                "woz": woz,
                "wow": wow,
                "dvw": dvw,
                "dvwb": dvwb,
                "idf": _IDF,
                "idb": _IDB,
            }
        )
    return in_maps


def _run(inputs, t_steps=T_STEPS, trace=False, reps=1, **kwargs):
    nc = _get_nc(t_steps, reps)
    in_maps = _prep_in_maps(inputs, t_steps)
    res = run_bass_kernel_spmd(nc, in_maps, list(range(N_CORES)), trace=trace, **kwargs)
    u_cores, x_cores = [], []
    for i in range(N_CORES):
        ou = np.asarray(res.results[i]["out_u"])            # [OUT, t, BC]
        u_cores.append(ou.transpose(2, 1, 0))               # [BC, t, OUT]
        ox = np.asarray(res.results[i]["out_x"])            # [NCH, STATE, CW]
        x_cores.append(ox.transpose(0, 2, 1).reshape(BC, STATE))
    u = np.concatenate(u_cores, axis=0)
    x = np.concatenate(x_cores, axis=0)
    out = np.empty((B, t_steps, 2 * OUT), np.float32)
    out[..., :OUT] = u
    out[..., OUT:] = np.float32(LOG_STD)
    return (out, x), res


def kernel(**inputs):
    (out, x), _ = _run(inputs)
    return out, x
